# revision 35
# baseline (speedup 1.0000x reference)
"""DenseKAN forward as a single fused matmul on TRN2.

Math: the reference's uniform knot grid gives cardinal cubic B-splines
B_j(x) = Q(u - j) with u = 2.5x + 5.5 in [3, 8).  In the truncated-power
form Q(s) = (1/6) sum_m (-1)^m C(4,m) relu(s-m)^3, every knot k <= 3
satisfies u >= k on the whole domain, so those terms are plain cubics in
x and collapse into the global polynomial {1, x, x^2, x^3}.  Only knots
k = 4..7 keep the relu:

    features per input dim: [x, x^2, x^3, silu(x), g4, g5, g6, g7]
    g_k(x) = relu(x + c_k)^3,  c_k = (5.5-k)/2.5 in {0.6, 0.2, -0.2, -0.6}

(7 spline features instead of 9; the constant feature plus the layer
bias is injected via one matmul against an all-ones stationary tile.)
Everything else folds into the weights on the host (float64), so the
layer is out = F(x) @ W with F computed on-chip in fp16:

    ACT: silu, q_k = Square(x + c_k) (fused bias)
    DVE: x^2 = x*x, x^3 = x^2*x, r_k = max(x+c_k, 0), g_k = q_k*r_k

The host ships x^T in fp16 directly into the feature tile's x block, so
the PE stream (which runs at the 1.2 GHz HAM mid-state for a kernel
this short) starts the moment x lands.  fp16 keeps the DVE in its 2x/4x
packed modes, halves the DMA, and holds quantization to ~5e-3 (bf16's 8
mantissa bits gave 3.7e-2).  GpSimd does nothing but tiny memsets (its
tensor_scalar runs ~15x below DVE here).  A throwaway Silu on a const
tile runs first on ACT so the activation-table load (silu_and_others,
which also covers Square and Copy -> one load total) hides inside the
DMA wait.  Batch is sharded across the 8 cores; weights replicated.
"""

import math

import numpy as np

import concourse.bass as bass
import concourse.mybir as mybir
import concourse.tile as tile
from concourse import bacc
from concourse.bass_utils import run_bass_kernel_spmd

BATCH = 1024
IN = 256
UNITS = 256
N_CORES = 8
BS = BATCH // N_CORES  # 128 batch rows per core

NB = 7  # T feature blocks: x2, x3, silu, g4..g7 (x + its weights ride xt_d)
KT = 13  # 12 feature k-tiles + bias slot (x, x^2 weights ride xt_d)
CS = (0.6, 0.2, -0.2, -0.6)  # biases for g4..g7
N_WARM = 1  # PE warm-up matmul (absorb the cold-start p-state step)
W_CHUNKS = ((0, 4), (4, 10), (10, 13))

FP32 = mybir.dt.float32
FP16 = mybir.dt.float16
AluOp = mybir.AluOpType
Act = mybir.ActivationFunctionType

_cache = {}


def _build():
    nc = bacc.Bacc("TRN2", target_bir_lowering=False, debug=False,
                   enable_asserts=False, num_devices=N_CORES)
    # [x^T (two dim-halves) | x- and x^2-weight k-tiles]: one DMA
    # delivers everything the first four matmuls need
    xt_d = nc.dram_tensor("xt", [128, 2 * BS + 4 * UNITS], FP16,
                          kind="ExternalInput").ap()
    # w[p, k, o] = feature k-tile k+4; w[p, 12, o] = bias_o (all rows equal)
    w_d = nc.dram_tensor("w2", [128, KT, UNITS], FP16, kind="ExternalInput").ap()
    o_d = nc.dram_tensor("out", [BS, UNITS], FP16, kind="ExternalOutput").ap()

    with tile.TileContext(nc) as tc:
        with (
            tc.tile_pool(name="const", bufs=1) as cpool,
            tc.tile_pool(name="psum", bufs=1, space="PSUM") as ppool,
        ):
            w2 = cpool.tile([128, KT, UNITS], FP16)
            wt = cpool.tile([128, 512], FP16)
            xw = cpool.tile([128, 2 * BS + 4 * UNITS], FP16)
            T = cpool.tile([128, NB * 256], FP16)
            qp = [cpool.tile([128, 512], FP16, name=f"q{m}")
                  for m in range(2)]
            rp = [cpool.tile([128, 512], FP16, name=f"r{m}")
                  for m in range(2)]
            osb = cpool.tile([BS, UNITS], FP16)
            wpsum = ppool.tile([128, 512], FP32)
            opsum = ppool.tile([BS, UNITS], FP32)

            blk = [T[:, b * 256:(b + 1) * 256] for b in range(NB)]
            xT = xw[:, 0:256]  # x lands here straight off the DMA

            # x + the x-block weights in one DMA at the head of every
            # queue: its single completion event releases the first two
            # matmuls (lhsT and rhs both live in xw)
            nc.sync.dma_start(xw[:], xt_d[:])
            for lo, hi in W_CHUNKS:
                nc.sync.dma_start(w2[:, lo:hi, :], w_d[:, lo:hi, :])

            # all-ones tile: warm-up fodder for the PE p-state step
            nc.vector.memset(wt[:], 1.0)
            for _ in range(N_WARM):
                nc.tensor.matmul(wpsum[:], wt[:, 0:128], wt[:],
                                 start=True, stop=True)

            def mm(b, stop=False):  # T block b >= 1; w slots shifted by 2
                for h in range(2):
                    k = 2 * b + h
                    nc.tensor.matmul(opsum[:], T[:, k * 128:(k + 1) * 128],
                                     w2[:, k - 2, :], start=False,
                                     stop=stop and h == 1)

            # ACT queue: just silu (its act-table load precedes it; both
            # finish ~2us before the silu-block matmuls need the data)
            nc.scalar.activation(blk[2], xT, Act.Silu)

            # DVE queue (fp16 2x/4x modes), interleaved with PE consumption;
            # g_k = relu(x+c)^3 as r*r*r keeps the whole chain on DVE
            for h in range(2):  # x block: stationary and weights off one DMA
                nc.tensor.matmul(opsum[:], xw[:, h * 128:(h + 1) * 128],
                                 xw[:, 256 + h * 256:512 + h * 256],
                                 start=h == 0, stop=False)
            nc.vector.tensor_mul(blk[0], xT, xT)        # x^2
            for h in range(2):  # x^2 block: weights also off the head DMA
                nc.tensor.matmul(opsum[:], T[:, h * 128:(h + 1) * 128],
                                 xw[:, 768 + h * 256:1024 + h * 256],
                                 start=False, stop=False)
            nc.vector.tensor_mul(blk[1], blk[0], xT)    # x^3
            mm(1)
            mm(2)                                       # silu block
            for p in range(2):
                for h in range(2):
                    nc.vector.tensor_scalar(rp[p][:, h * 256:(h + 1) * 256],
                                            xT, float(CS[2 * p + h]), 0.0,
                                            AluOp.add, AluOp.max)
                nc.vector.tensor_mul(qp[p][:], rp[p][:], rp[p][:])
                nc.vector.tensor_mul(T[:, (3 + 2 * p) * 256:(5 + 2 * p) * 256],
                                     qp[p][:], rp[p][:])
                mm(3 + 2 * p)
                mm(4 + 2 * p, stop=p == 1)

            # drain: the bias lives in w2 slot 12 (every row = bias_o), so
            # the PSUM->SBUF move doubles as the bias add; then both rings
            # issue a half-DMA each
            nc.vector.tensor_add(osb[:], opsum[:], w2[:, 12, :])
            nc.sync.dma_start(o_d[:, 0:128], osb[:, 0:128])
            nc.scalar.dma_start(o_d[:, 128:256], osb[:, 128:256])

    nc.compile()
    return nc


def _coef_matrices():
    """P[j,d]: coeff of x^d in B_j's polynomial part; R[j,m]: coeff of g_{4+m}."""
    P = np.zeros((8, 4))
    R = np.zeros((8, 4))
    for j in range(8):
        for m in range(5):
            k = j + m
            s = (-1) ** m * math.comb(4, m) / 6.0
            if k <= 3:
                for d in range(4):
                    P[j, d] += s * math.comb(3, d) * 2.5 ** d * (5.5 - k) ** (3 - d)
            elif k <= 7:
                R[j, k - 4] += s * 2.5 ** 3
    return P, R


def _fold_weights(spline_kernel, scale_factor, bias):
    """-> (wx (128, 1024) x/x^2-weight k-tiles, (128, KT, UNITS) rest)."""
    sk = spline_kernel.astype(np.float64)
    sf = scale_factor.astype(np.float64)
    b = bias.astype(np.float64)
    V = sk * sf[:, None, :]  # (in, 8, out)
    P, R = _coef_matrices()
    Wpoly = np.einsum("jd,ijo->dio", P, V)  # const, x, x^2, x^3
    Wg = np.einsum("jm,ijo->mio", R, V)  # g4..g7
    c = b + Wpoly[0].sum(axis=0)  # (out,)
    feats = np.concatenate([Wpoly[1:], sf[None], Wg], axis=0)  # (8, in, out)
    kt = feats.reshape(16, 128, UNITS)  # k-tile k, dim row p
    wx = np.ascontiguousarray(
        kt[0:4].transpose(1, 0, 2).reshape(128, 4 * UNITS)
        .astype(np.float32).astype(np.float16))
    full = np.concatenate([kt[4:], np.broadcast_to(c, (1, 128, UNITS))], 0)
    sw = full.transpose(1, 0, 2)  # -> [p, slot, o]
    return wx, np.ascontiguousarray(sw.astype(np.float32).astype(np.float16))


def _prep_x(x, wx):
    """(BATCH, IN) -> per-core (128, 2*BS + 1024) fp16 [x^T halves | wx]."""
    x = np.asarray(x, dtype=np.float32)
    outs = []
    for cid in range(N_CORES):
        xtc = np.ascontiguousarray(x[cid * BS:(cid + 1) * BS].T)  # (IN, BS)
        xh = np.concatenate([xtc[:128], xtc[128:]], axis=1).astype(np.float16)
        outs.append(np.ascontiguousarray(np.concatenate([xh, wx], axis=1)))
    return outs


def kernel(x, spline_kernel, scale_factor, bias):
    if "nc" not in _cache:
        _cache["nc"] = _build()
    nc = _cache["nc"]

    wx, w2 = _fold_weights(spline_kernel, scale_factor, bias)
    xts = _prep_x(x, wx)
    in_maps = [{"xt": xts[c], "w2": w2} for c in range(N_CORES)]
    res = run_bass_kernel_spmd(nc, in_maps, list(range(N_CORES)))
    out = np.concatenate([res.results[c]["out"] for c in range(N_CORES)],
                         axis=0)
    return out.astype(np.float32)


# revision 36
# speedup vs baseline: 1.0280x; 1.0280x over previous
"""DenseKAN forward as a single fused matmul on TRN2.

Math: the reference's uniform knot grid gives cardinal cubic B-splines
B_j(x) = Q(u - j) with u = 2.5x + 5.5 in [3, 8).  In the truncated-power
form Q(s) = (1/6) sum_m (-1)^m C(4,m) relu(s-m)^3, every knot k <= 3
satisfies u >= k on the whole domain, so those terms are plain cubics in
x and collapse into the global polynomial {1, x, x^2, x^3}.  Only knots
k = 4..7 keep the relu:

    features per input dim: [x, x^2, x^3, silu(x), g4, g5, g6, g7]
    g_k(x) = relu(x + c_k)^3,  c_k = (5.5-k)/2.5 in {0.6, 0.2, -0.2, -0.6}

(7 spline features instead of 9; the constant feature plus the layer
bias is injected via one matmul against an all-ones stationary tile.)
Everything else folds into the weights on the host (float64), so the
layer is out = F(x) @ W with F computed on-chip in fp16:

    ACT: silu, q_k = Square(x + c_k) (fused bias)
    DVE: x^2 = x*x, x^3 = x^2*x, r_k = max(x+c_k, 0), g_k = q_k*r_k

The host ships x^T in fp16 directly into the feature tile's x block, so
the PE stream (which runs at the 1.2 GHz HAM mid-state for a kernel
this short) starts the moment x lands.  fp16 keeps the DVE in its 2x/4x
packed modes, halves the DMA, and holds quantization to ~5e-3 (bf16's 8
mantissa bits gave 3.7e-2).  GpSimd does nothing but tiny memsets (its
tensor_scalar runs ~15x below DVE here).  A throwaway Silu on a const
tile runs first on ACT so the activation-table load (silu_and_others,
which also covers Square and Copy -> one load total) hides inside the
DMA wait.  Batch is sharded across the 8 cores; weights replicated.
"""

import math

import numpy as np

import concourse.bass as bass
import concourse.mybir as mybir
import concourse.tile as tile
from concourse import bacc
from concourse.bass_utils import run_bass_kernel_spmd

BATCH = 1024
IN = 256
UNITS = 256
N_CORES = 8
BS = BATCH // N_CORES  # 128 batch rows per core

NB = 7  # T feature blocks: x2, x3, silu, g4..g7 (x + its weights ride xt_d)
KT = 15  # 14 feature k-tiles + bias slot
CS = (0.6, 0.2, -0.2, -0.6)  # biases for g4..g7
N_WARM = 1  # PE warm-up matmul (absorb the cold-start p-state step)
W_CHUNKS = ((0, 6), (6, 12), (12, 15))

FP32 = mybir.dt.float32
FP16 = mybir.dt.float16
AluOp = mybir.AluOpType
Act = mybir.ActivationFunctionType

_cache = {}


def _build():
    nc = bacc.Bacc("TRN2", target_bir_lowering=False, debug=False,
                   enable_asserts=False, num_devices=N_CORES)
    # [x^T (two dim-halves) | x-feature weight k-tiles 0,1]: one DMA
    # delivers everything the first two matmuls need
    xt_d = nc.dram_tensor("xt", [128, 2 * BS + 2 * UNITS], FP16,
                          kind="ExternalInput").ap()
    # w[p, k, o] = feature k-tile k+2; w[p, 14, o] = bias_o (all rows equal)
    w_d = nc.dram_tensor("w2", [128, KT, UNITS], FP16, kind="ExternalInput").ap()
    o_d = nc.dram_tensor("out", [BS, UNITS], FP16, kind="ExternalOutput").ap()

    with tile.TileContext(nc) as tc:
        with (
            tc.tile_pool(name="const", bufs=1) as cpool,
            tc.tile_pool(name="psum", bufs=1, space="PSUM") as ppool,
        ):
            w2 = cpool.tile([128, KT, UNITS], FP16)
            wt = cpool.tile([128, 512], FP16)
            xw = cpool.tile([128, 2 * BS + 2 * UNITS], FP16)
            T = cpool.tile([128, NB * 256], FP16)
            qp = [cpool.tile([128, 512], FP16, name=f"q{m}")
                  for m in range(2)]
            rp = [cpool.tile([128, 512], FP16, name=f"r{m}")
                  for m in range(2)]
            osb = cpool.tile([BS, UNITS], FP16)
            wpsum = ppool.tile([128, 512], FP32)
            opsum = ppool.tile([BS, UNITS], FP32)

            blk = [T[:, b * 256:(b + 1) * 256] for b in range(NB)]
            xT = xw[:, 0:256]  # x lands here straight off the DMA

            # x + the x-block weights in one DMA at the head of every
            # queue: its single completion event releases the first two
            # matmuls (lhsT and rhs both live in xw)
            nc.sync.dma_start(xw[:], xt_d[:])
            for lo, hi in W_CHUNKS:
                nc.sync.dma_start(w2[:, lo:hi, :], w_d[:, lo:hi, :])

            # all-ones tile: warm-up fodder for the PE p-state step
            nc.vector.memset(wt[:], 1.0)
            for _ in range(N_WARM):
                nc.tensor.matmul(wpsum[:], wt[:, 0:128], wt[:],
                                 start=True, stop=True)

            def mm(b, stop=False):
                for h in range(2):
                    k = 2 * b + h
                    nc.tensor.matmul(opsum[:], T[:, k * 128:(k + 1) * 128],
                                     w2[:, k, :], start=False,
                                     stop=stop and h == 1)

            # ACT queue: just silu (its act-table load precedes it; both
            # finish ~2us before the silu-block matmuls need the data)
            nc.scalar.activation(blk[2], xT, Act.Silu)

            # DVE queue (fp16 2x/4x modes), interleaved with PE consumption;
            # g_k = relu(x+c)^3 as r*r*r keeps the whole chain on DVE
            for h in range(2):  # x block: stationary and weights off one DMA
                nc.tensor.matmul(opsum[:], xw[:, h * 128:(h + 1) * 128],
                                 xw[:, 256 + h * 256:512 + h * 256],
                                 start=h == 0, stop=False)
            nc.vector.tensor_mul(blk[0], xT, xT)        # x^2
            mm(0)
            nc.vector.tensor_mul(blk[1], blk[0], xT)    # x^3
            mm(1)
            mm(2)                                       # silu block
            for p in range(2):
                for h in range(2):
                    nc.vector.tensor_scalar(rp[p][:, h * 256:(h + 1) * 256],
                                            xT, float(CS[2 * p + h]), 0.0,
                                            AluOp.add, AluOp.max)
                nc.vector.tensor_mul(qp[p][:], rp[p][:], rp[p][:])
                nc.vector.tensor_mul(T[:, (3 + 2 * p) * 256:(5 + 2 * p) * 256],
                                     qp[p][:], rp[p][:])
                mm(3 + 2 * p)
                mm(4 + 2 * p, stop=p == 1)

            # drain: the bias lives in w2 slot 14 (every row = bias_o), so
            # the PSUM->SBUF move doubles as the bias add; then both rings
            # issue a half-DMA each
            nc.vector.tensor_add(osb[:], opsum[:], w2[:, 14, :])
            nc.sync.dma_start(o_d[:, 0:128], osb[:, 0:128])
            nc.scalar.dma_start(o_d[:, 128:256], osb[:, 128:256])

    nc.compile()
    return nc


def _coef_matrices():
    """P[j,d]: coeff of x^d in B_j's polynomial part; R[j,m]: coeff of g_{4+m}."""
    P = np.zeros((8, 4))
    R = np.zeros((8, 4))
    for j in range(8):
        for m in range(5):
            k = j + m
            s = (-1) ** m * math.comb(4, m) / 6.0
            if k <= 3:
                for d in range(4):
                    P[j, d] += s * math.comb(3, d) * 2.5 ** d * (5.5 - k) ** (3 - d)
            elif k <= 7:
                R[j, k - 4] += s * 2.5 ** 3
    return P, R


def _fold_weights(spline_kernel, scale_factor, bias):
    """-> (wx (128, 512) x-weight k-tiles, (128, KT, UNITS) rest+bias)."""
    sk = spline_kernel.astype(np.float64)
    sf = scale_factor.astype(np.float64)
    b = bias.astype(np.float64)
    V = sk * sf[:, None, :]  # (in, 8, out)
    P, R = _coef_matrices()
    Wpoly = np.einsum("jd,ijo->dio", P, V)  # const, x, x^2, x^3
    Wg = np.einsum("jm,ijo->mio", R, V)  # g4..g7
    c = b + Wpoly[0].sum(axis=0)  # (out,)
    feats = np.concatenate([Wpoly[1:], sf[None], Wg], axis=0)  # (8, in, out)
    kt = feats.reshape(16, 128, UNITS)  # k-tile k, dim row p
    wx = np.ascontiguousarray(
        kt[0:2].transpose(1, 0, 2).reshape(128, 2 * UNITS)
        .astype(np.float32).astype(np.float16))
    full = np.concatenate([kt[2:], np.broadcast_to(c, (1, 128, UNITS))], 0)
    sw = full.transpose(1, 0, 2)  # -> [p, slot, o]
    return wx, np.ascontiguousarray(sw.astype(np.float32).astype(np.float16))


def _prep_x(x, wx):
    """(BATCH, IN) -> per-core (128, 2*BS + 512) fp16 [x^T halves | wx]."""
    x = np.asarray(x, dtype=np.float32)
    outs = []
    for cid in range(N_CORES):
        xtc = np.ascontiguousarray(x[cid * BS:(cid + 1) * BS].T)  # (IN, BS)
        xh = np.concatenate([xtc[:128], xtc[128:]], axis=1).astype(np.float16)
        outs.append(np.ascontiguousarray(np.concatenate([xh, wx], axis=1)))
    return outs


def kernel(x, spline_kernel, scale_factor, bias):
    if "nc" not in _cache:
        _cache["nc"] = _build()
    nc = _cache["nc"]

    wx, w2 = _fold_weights(spline_kernel, scale_factor, bias)
    xts = _prep_x(x, wx)
    in_maps = [{"xt": xts[c], "w2": w2} for c in range(N_CORES)]
    res = run_bass_kernel_spmd(nc, in_maps, list(range(N_CORES)))
    out = np.concatenate([res.results[c]["out"] for c in range(N_CORES)],
                         axis=0)
    return out.astype(np.float32)
